# revision 1
# baseline (speedup 1.0000x reference)
"""AMICO ADMM solver on 8 TRN2 NeuronCores.

min_x ||y - A x||^2 + lambda*|x|_1, x >= 0 via ADMM (100 iterations),
data-parallel over voxels (1024 voxels per core).

Algebraic restructuring (rho=1, kappa=lambda/rho):
  Reference per-iteration:
    rhs = AtY + (z - u); x = W @ rhs; v = x + u
    z' = relu(v - kappa); u' = min(v, kappa)
  With s' := (z - u) + kappa = |v - kappa|, m := u = min(v, kappa),
  B := W @ AtY2 (constant, AtY2 = AtY + kappa*(AtA @ 1)), D := B - kappa:
    psum = W @ s' + D             # 8 fp16 matmuls + 4 identity-inject matmuls
                                  # (psum == x' - kappa; D injected via I @ D)
    v    = psum + m               # VectorE tensor_tensor (PSUM + SBUF)
    s'   = |v - kappa|            # ScalarE Abs activation -> fp16
    m    = min(v, kappa)          # VectorE tensor_scalar (fp16 4x perf mode)
  Final output: x_100 = psum_100 directly.
"""

import os

import numpy as np

M = 256
K = 256
N_VOX = 8192
N_CORES = 8
N_SHARD = N_VOX // N_CORES  # 1024
RHO = 1.0
LAMBDA_REG = 0.1
KAPPA = LAMBDA_REG / RHO
N_ITERS = 100

LAST_RESULTS = None  # BassKernelResults of the most recent run (for test.py)


def _build_graph():
    import concourse.mybir as mybir
    from concourse import bacc
    from concourse.tile import TileContext

    f32 = mybir.dt.float32
    f32r = mybir.dt.float32r
    fp16 = mybir.dt.float16
    kap = float(KAPPA)

    nc = bacc.Bacc("TRN2", target_bir_lowering=False, debug=False)

    # D16[p, r*1024 + n] = D[r*128+p, n],  D = W@AtY - kappa*(W@1) (host f64)
    D16_p = nc.declare_dram_parameter("D16", [128, 2048], fp16, isOutput=False)
    I_p = nc.declare_dram_parameter("ident", [128, 128], fp16, isOutput=False)
    W16_p = nc.declare_dram_parameter("W16", [128, 512], fp16, isOutput=False)
    O_p = nc.declare_dram_parameter("out", [128, 2048], f32, isOutput=True)

    absf = mybir.ActivationFunctionType.Abs

    with TileContext(nc) as tc:
        with (
            tc.tile_pool(name="static", bufs=1) as statics,
            tc.tile_pool(name="spool", bufs=8) as spool,
            tc.tile_pool(name="vpool", bufs=8) as vpool,
            tc.tile_pool(name="mpool", bufs=8) as mpool,
        ):
            D16_sb = statics.tile([128, 2048], fp16, name="D16_sb")
            nc.sync.dma_start(D16_sb[:, :], D16_p[:, :])
            i_sb = statics.tile([128, 128], fp16, name="i_sb")
            nc.sync.dma_start(i_sb[:, :], I_p[:, :])
            W16_sb = statics.tile([128, 512], fp16, name="W16_sb")
            nc.sync.dma_start(W16_sb[:, :], W16_p[:, :])
            out_sb = statics.tile([128, 2048], f32, name="out_sb")
            nkapb_sb = statics.tile([128, 1], f32, name="nkapb_sb")
            nc.vector.memset(nkapb_sb[:, :], -kap)
            kconst = statics.tile([128, 512], f32, name="kconst")
            nc.vector.memset(kconst[:, :], kap)

            warm_sb = statics.tile([1, 8], f32, name="warm_sb")
            nc.scalar.activation(
                warm_sb[:, :], nkapb_sb[:1, :].to_broadcast((1, 8)), absf,
                bias=nkapb_sb[:1, :], scale=1.0,
            )

            s_h = [[None, None], [None, None]]
            m_c = [None, None]  # [128,1024] per column: [h0 | h1]
            for h in (0, 1):
                for c in (0, 1):
                    s0 = spool.tile([128, 512], fp16, name="s_new", tag="s")
                    nc.vector.tensor_copy(s0[:, :], kconst[:, :])
                    s_h[h][c] = s0
            for c in (0, 1):
                m0 = mpool.tile([128, 1024], fp16, name="m_new", tag="m")
                nc.vector.memset(m0[:, :], 0.0)
                m_c[c] = m0

            with tc.tile_pool(name="psum_loop", bufs=8, space="PSUM") as psl:
                for it in range(N_ITERS):
                    last = it == N_ITERS - 1
                    ps_rc = [[None, None], [None, None]]
                    for c in (0, 1):
                        for r in (0, 1):
                            ps = psl.tile([128, 512], f32, name="ps_x", tag="ps")
                            d0 = r * 1024 + c * 512
                            nc.tensor.matmul(
                                ps[:, :],
                                i_sb[:, :],
                                D16_sb[:, d0 : d0 + 512],
                                start=True,
                                stop=False,
                                skip_group_check=True,
                            )
                            ps_rc[r][c] = ps
                    for c in (0, 1):
                        for r in (0, 1):
                            for kc in (0, 1):
                                w0 = kc * 256 + r * 128
                                nc.tensor.matmul(
                                    ps_rc[r][c][:, :],
                                    W16_sb[:, w0 : w0 + 128],
                                    s_h[kc][c][:, :],
                                    start=False,
                                    stop=(kc == 1),
                                    skip_group_check=True,
                                )

                    if last:
                        for h in (0, 1):
                            for c in (0, 1):
                                sl = slice(
                                    h * 1024 + c * 512, h * 1024 + c * 512 + 512
                                )
                                nc.scalar.copy(out_sb[:, sl], ps_rc[h][c][:, :])
                                nc.sync.dma_start(O_p[:, sl], out_sb[:, sl])
                        break

                    new_s = [[None, None], [None, None]]
                    new_m = [None, None]
                    for c in (0, 1):
                        # v for both h-halves into ONE [128,1024] tile so the
                        # min becomes a single wide 4x-mode tensor_scalar.
                        vc = vpool.tile([128, 1024], fp16, name="v", tag="v")
                        for h in (0, 1):
                            hs = slice(h * 512, h * 512 + 512)
                            nc.vector.tensor_add(
                                vc[:, hs], ps_rc[h][c][:, :], m_c[c][:, hs]
                            )
                        mn = mpool.tile([128, 1024], fp16, name="m_new", tag="m")
                        nc.vector.tensor_scalar_min(mn[:, :], vc[:, :], kap)
                        new_m[c] = mn
                        for h in (0, 1):
                            hs = slice(h * 512, h * 512 + 512)
                            sn = spool.tile([128, 512], fp16, name="s_new", tag="s")
                            nc.scalar.activation(
                                sn[:, :], vc[:, hs], absf,
                                bias=nkapb_sb[:, :], scale=1.0,
                            )
                            new_s[h][c] = sn
                    s_h, m_c = new_s, new_m



    nc.compile()
    return nc


_GRAPH = None


def kernel(A: np.ndarray, data: np.ndarray) -> np.ndarray:
    global _GRAPH, LAST_RESULTS
    from concourse.bass_utils import run_bass_kernel_spmd

    A = np.ascontiguousarray(np.asarray(A, dtype=np.float32))
    data = np.ascontiguousarray(np.asarray(data, dtype=np.float32))
    assert A.shape == (M, K) and data.shape == (N_VOX, M)

    A64 = A.astype(np.float64)
    AtA = A64.T @ A64
    W = np.linalg.inv(AtA + RHO * np.eye(K))
    w1 = KAPPA * (W @ np.ones(K))

    W_dev = (
        W.astype(np.float32).reshape(2, 128, K).transpose(1, 0, 2).reshape(128, 2 * K)
    )
    i_dev = np.eye(128, dtype=np.float16)
    W16_dev = W_dev.astype(np.float16)

    in_maps = []
    for i in range(N_CORES):
        shard = data[i * N_SHARD : (i + 1) * N_SHARD]  # [1024, 256]
        AtY = A64.T @ shard.astype(np.float64).T  # [256, 1024]
        D = (W @ AtY) - w1[:, None]  # [256, 1024] f64
        D_dev = (
            D.astype(np.float16)
            .reshape(2, 128, N_SHARD)
            .transpose(1, 0, 2)
            .reshape(128, 2 * N_SHARD)
        )
        in_maps.append(
            {
                "D16": np.ascontiguousarray(D_dev),
                "ident": i_dev,
                "W16": W16_dev,
            }
        )

    if _GRAPH is None:
        _GRAPH = _build_graph()

    trace = bool(int(os.environ.get("KERNEL_TRACE", "0")))
    res = run_bass_kernel_spmd(
        _GRAPH, in_maps, core_ids=list(range(N_CORES)), trace=trace
    )
    LAST_RESULTS = res

    out = np.empty((N_VOX, K), dtype=np.float32)
    for i in range(N_CORES):
        o = res.results[i]["out"]  # [128, 2048]
        X = o.reshape(128, 2, N_SHARD).transpose(1, 0, 2).reshape(K, N_SHARD)
        out[i * N_SHARD : (i + 1) * N_SHARD] = X.T
    return out

